# revision 5
# baseline (speedup 1.0000x reference)
"""Row L2-normalization kernel for Trainium2 (raw Bass), 8-core SPMD.

out[i, j] = corr[i, j] / sqrt(sum_j corr[i, j]^2)

Sharding: row-wise across 8 cores — each core owns a [1024, 8192] slab.
Row norms are fully row-local, so there is no cross-core communication.

The shipped build is `_build_nc_v2` (row-pair layout): a pass is 4
pair-tiles of 256 rows; partition p of pair-tile j holds DRAM rows
256j+2p and 256j+2p+1, so load lines are 64 KB contiguous DRAM per
partition (max-size SDMA descriptors) and each pass moves 4 loads of
8 MB + 4 stores of 4 MB. One engine per pipeline stage:

    SP   : DMA load x -> t[j%2]            (8 MB pair-loads)
    ACT  : per 8192-col sub-block: Square(out=junk, accum_out=rowsum);
           Sqrt(rowsum)
    DVE  : per sub-block: reciprocal(rowsum); o = t * rowsum
           (tensor_scalar_mul, f32 -> bf16 downcast on write)
    POOL : DMA store o -> y                (4 MB pair-stores)

The output is written to HBM as bf16 and widened to f32 on the host
(exact widening; the only loss is the bf16 rounding of the normalized
values — the normalized entries are all in [-1, 1] so bf16's 8-bit
mantissa bounds the per-element relative error at 2^-9; measured
Frobenius rel err 1.66e-3 vs the 2e-2 gate). This cuts per-core HBM
traffic from 64 MB (32 read + 32 write f32) to 48 MB (32 read f32 +
16 write bf16).

Measured on HW via K-pass NEFFs (see test.py): 158.3-159.1 us/pass,
vs ~200 us for the f32-out predecessor. A DMA-only probe with the
identical transfer shapes and no compute measured 157.6 us/pass, i.e.
this kernel runs at the pure DMA floor for its 2:1 read:write mix
(~319 GB/s/core effective of the 360 GB/s/core spec) with compute and
sync fully hidden. The remaining byte count is also floored: the f32
input must be read as-is, and a 1-byte output dtype (e4m3, ~3% RMS
quantization) would breach the 2e-2 gate.

`_build_nc*(n_passes=K)` emits K back-to-back full passes in one NEFF
(every pass re-reads x from HBM and re-writes y); the sliding-window
semaphore formulas are uniform in the global tile index so passes
pipeline seamlessly. kernel() itself uses the 1-pass build; test.py
uses a K-pass build to measure per-pass device time with the dispatch
overhead amortized away.

`_build_nc` (v1, kept for reference) is the same pipeline with 8
single-tile transfers per pass ([128, 8192] tiles, 3-deep buffers);
it measured 159.6-160.4 us/pass — the pair layout buys ~1.3 us.

Raw Bass (not Tile) because this walrus build rejects compute
instructions carrying >1 embedded semaphore wait; here every wait is a
standalone wait_ge.
"""

import sys

for _p in ("/opt/trn_rl_repo", "/root/.axon_site/_ro/trn_rl_repo"):
    if _p not in sys.path:
        sys.path.append(_p)

import numpy as np

DIM = 8192
N_CORES = 8
ROWS_PER_CORE = DIM // N_CORES  # 1024
P = 128
N_TILES = ROWS_PER_CORE // P  # 8
N_T_BUFS = 3
N_O_BUFS = 3

_CACHE: dict = {}


def _build_nc(n_passes: int = 1):
    import concourse.bass as bass
    from concourse import mybir

    nc = bass.Bass()
    f32 = mybir.dt.float32
    bf16 = mybir.dt.bfloat16
    x = nc.dram_tensor("x", [ROWS_PER_CORE, DIM], f32, kind="ExternalInput")
    y = nc.dram_tensor("y", [ROWS_PER_CORE, DIM], bf16, kind="ExternalOutput")
    xt = x.rearrange("(n p) m -> n p m", p=P)
    yt = y.rearrange("(n p) m -> n p m", p=P)

    TOTAL = n_passes * N_TILES

    with (
        nc.sbuf_tensor([P, N_T_BUFS, DIM], f32) as t_buf,
        nc.sbuf_tensor([P, N_O_BUFS, DIM], bf16) as o_buf,
        nc.sbuf_tensor([P, N_TILES], f32) as norms,
        nc.semaphore("t_sem0") as t_sem0,
        nc.semaphore("t_sem1") as t_sem1,
        nc.semaphore("t_sem2") as t_sem2,
        nc.semaphore("o_sem0") as o_sem0,
        nc.semaphore("o_sem1") as o_sem1,
        nc.semaphore("o_sem2") as o_sem2,
        nc.semaphore("act") as act_sem,
        nc.semaphore("dve") as dve_sem,
        nc.Block() as block,
    ):
        # One DMA semaphore per buffer slot: a DMA's 16 increments land
        # unordered across SDMA engines, so cumulative waits on a sem shared
        # by concurrent DMAs would be racy. Per slot, transfers serialize.
        t_sems = [t_sem0, t_sem1, t_sem2]
        o_sems = [o_sem0, o_sem1, o_sem2]

        @block.sync
        def _(sync):
            for i in range(TOTAL):
                if i >= N_T_BUFS:
                    # t-slot free once the DVE scale of tile i-3 has read it
                    sync.wait_ge(dve_sem, 2 * (i - N_T_BUFS) + 2)
                sync.dma_start(
                    out=t_buf[:, i % N_T_BUFS, :], in_=xt[i % N_TILES]
                ).then_inc(t_sems[i % N_T_BUFS], 16)

        @block.scalar
        def _(scalar):
            for i in range(TOTAL):
                t = t_buf[:, i % N_T_BUFS, :]
                o = o_buf[:, i % N_O_BUFS, :]
                norm = norms[:, i % N_TILES : i % N_TILES + 1]
                scalar.wait_ge(t_sems[i % N_T_BUFS], 16 * (i // N_T_BUFS + 1))
                if i >= N_O_BUFS:
                    # o-slot free once tile i-3's store has drained
                    scalar.wait_ge(o_sems[i % N_O_BUFS], 16 * (i // N_O_BUFS))
                # The Square's elementwise output is junk dumped into the
                # o-tile (the DVE scale overwrites it); only accum_out is
                # consumed.
                scalar.activation(
                    out=o,
                    in_=t,
                    func=mybir.ActivationFunctionType.Square,
                    accum_out=norm,
                ).then_inc(act_sem, 1)
                # ACT pipelines back-to-back instructions; the accum_out
                # write lands at completion, so same-engine RAW needs a wait.
                scalar.wait_ge(act_sem, 2 * i + 1)
                scalar.sqrt(out=norm, in_=norm).then_inc(act_sem, 1)

        HALF = DIM // 2
        LAST = TOTAL - 1

        @block.vector
        def _(vector):
            for i in range(TOTAL):
                t = t_buf[:, i % N_T_BUFS, :]
                o = o_buf[:, i % N_O_BUFS, :]
                norm = norms[:, i % N_TILES : i % N_TILES + 1]
                # sqrt done => square done => load i landed (sem values fire
                # at instruction completion, so this transitivity is sound)
                vector.wait_ge(act_sem, 2 * i + 2)
                vector.reciprocal(out=norm, in_=norm).then_inc(dve_sem, 1)
                vector.wait_ge(dve_sem, 2 * i + 1)
                if i < LAST:
                    vector.tensor_scalar_mul(o, t, norm).then_inc(dve_sem, 1)
                else:
                    # Last tile: scale in column halves so the first half-
                    # store overlaps the second half-scale (shorter tail).
                    vector.tensor_scalar_mul(
                        o[:, :HALF], t[:, :HALF], norm
                    ).then_inc(dve_sem, 1)
                    vector.wait_ge(dve_sem, 2 * i + 2)
                    vector.tensor_scalar_mul(
                        o[:, HALF:], t[:, HALF:], norm
                    ).then_inc(dve_sem, 1)

        @block.gpsimd
        def _(gpsimd):
            for i in range(TOTAL):
                o = o_buf[:, i % N_O_BUFS, :]
                yto = yt[i % N_TILES]
                gpsimd.wait_ge(dve_sem, 2 * i + 2)
                if i < LAST:
                    gpsimd.dma_start(out=yto, in_=o).then_inc(
                        o_sems[i % N_O_BUFS], 16
                    )
                else:
                    gpsimd.dma_start(
                        out=yto[:, :HALF], in_=o[:, :HALF]
                    ).then_inc(o_sems[i % N_O_BUFS], 16)
                    gpsimd.wait_ge(dve_sem, 2 * i + 3)
                    gpsimd.dma_start(
                        out=yto[:, HALF:], in_=o[:, HALF:]
                    ).then_inc(o_sems[i % N_O_BUFS], 16)

    return nc


def _build_nc_v2(n_passes: int = 1):
    """Row-pair layout: partition p of pair-tile j holds DRAM rows
    256*j + 2p and 256*j + 2p + 1, so each partition's load line is
    64 KB contiguous DRAM (one max-size SDMA descriptor) and each
    store line is 32 KB contiguous. A pass is 4 pair-tiles: 4 loads
    of 8 MB + 4 stores of 4 MB instead of 8+8 half-size transfers —
    half the per-transfer fixed costs (DGE issue + semaphore
    propagation) that dominate the gap to the bandwidth roofline.

    The two rows sharing a partition are normalized independently:
    the ACT Square/accum runs once per 8192-column sub-block, giving
    one row-norm per partition per sub-block.
    """
    import concourse.bass as bass
    from concourse import mybir

    nc = bass.Bass()
    f32 = mybir.dt.float32
    bf16 = mybir.dt.bfloat16
    x = nc.dram_tensor("x", [ROWS_PER_CORE, DIM], f32, kind="ExternalInput")
    y = nc.dram_tensor("y", [ROWS_PER_CORE, DIM], bf16, kind="ExternalOutput")
    # pair-tile j, partition p, sub-block s <-> DRAM row 256j + 2p + s
    xt = x.rearrange("(n p two) m -> n p (two m)", p=P, two=2)
    yt = y.rearrange("(n p two) m -> n p (two m)", p=P, two=2)

    PAIRS = N_TILES // 2  # 4 pair-tiles per pass
    W2 = 2 * DIM
    TOTAL = n_passes * PAIRS
    N_SLOTS = 2

    with (
        nc.sbuf_tensor([P, N_SLOTS, W2], f32) as t_buf,
        nc.sbuf_tensor([P, N_SLOTS, W2], bf16) as o_buf,
        nc.sbuf_tensor([P, 2 * PAIRS], f32) as norms,
        nc.semaphore("t_sem0") as t_sem0,
        nc.semaphore("t_sem1") as t_sem1,
        nc.semaphore("o_sem0") as o_sem0,
        nc.semaphore("o_sem1") as o_sem1,
        nc.semaphore("act") as act_sem,
        nc.semaphore("dve") as dve_sem,
        nc.Block() as block,
    ):
        t_sems = [t_sem0, t_sem1]
        o_sems = [o_sem0, o_sem1]
        LAST = TOTAL - 1

        @block.sync
        def _(sync):
            for j in range(TOTAL):
                if j >= N_SLOTS:
                    # t-slot free once both DVE scales of pair j-2 read it
                    sync.wait_ge(dve_sem, 4 * (j - N_SLOTS) + 4)
                sync.dma_start(
                    out=t_buf[:, j % N_SLOTS, :], in_=xt[j % PAIRS]
                ).then_inc(t_sems[j % N_SLOTS], 16)

        @block.scalar
        def _(scalar):
            for j in range(TOTAL):
                t = t_buf[:, j % N_SLOTS, :]
                o = o_buf[:, j % N_SLOTS, :]
                scalar.wait_ge(t_sems[j % N_SLOTS], 16 * (j // N_SLOTS + 1))
                if j >= N_SLOTS:
                    # o-slot free once pair j-2's store has drained
                    scalar.wait_ge(o_sems[j % N_SLOTS], 16 * (j // N_SLOTS))
                for s in (0, 1):
                    c = (j % PAIRS) * 2 + s
                    norm = norms[:, c : c + 1]
                    sub = slice(s * DIM, (s + 1) * DIM)
                    # Square's elementwise output is junk dumped into the
                    # o sub-block (the DVE scale overwrites it).
                    scalar.activation(
                        out=o[:, sub],
                        in_=t[:, sub],
                        func=mybir.ActivationFunctionType.Square,
                        accum_out=norm,
                    ).then_inc(act_sem, 1)
                    scalar.wait_ge(act_sem, 4 * j + 2 * s + 1)
                    scalar.sqrt(out=norm, in_=norm).then_inc(act_sem, 1)

        @block.vector
        def _(vector):
            for j in range(TOTAL):
                t = t_buf[:, j % N_SLOTS, :]
                o = o_buf[:, j % N_SLOTS, :]
                for s in (0, 1):
                    c = (j % PAIRS) * 2 + s
                    norm = norms[:, c : c + 1]
                    sub = slice(s * DIM, (s + 1) * DIM)
                    vector.wait_ge(act_sem, 4 * j + 2 * s + 2)
                    vector.reciprocal(out=norm, in_=norm).then_inc(dve_sem, 1)
                    vector.wait_ge(dve_sem, 4 * j + 2 * s + 1)
                    vector.tensor_scalar_mul(
                        o[:, sub], t[:, sub], norm
                    ).then_inc(dve_sem, 1)

        @block.gpsimd
        def _(gpsimd):
            for j in range(TOTAL):
                o = o_buf[:, j % N_SLOTS, :]
                yto = yt[j % PAIRS]
                if j < LAST:
                    gpsimd.wait_ge(dve_sem, 4 * j + 4)
                    gpsimd.dma_start(out=yto, in_=o).then_inc(
                        o_sems[j % N_SLOTS], 16
                    )
                else:
                    # Last pair: store per sub-block so the first store
                    # overlaps the second scale (shorter tail).
                    gpsimd.wait_ge(dve_sem, 4 * j + 2)
                    gpsimd.dma_start(
                        out=yto[:, :DIM], in_=o[:, :DIM]
                    ).then_inc(o_sems[j % N_SLOTS], 16)
                    gpsimd.wait_ge(dve_sem, 4 * j + 4)
                    gpsimd.dma_start(
                        out=yto[:, DIM:], in_=o[:, DIM:]
                    ).then_inc(o_sems[j % N_SLOTS], 16)

    return nc


def _get_nc():
    if "nc" not in _CACHE:
        _CACHE["nc"] = _build_nc_v2()
    return _CACHE["nc"]


def _make_callable(nc, donate: bool = True):
    """Compile a Bass module into a sharded PJRT callable over 8 cores.

    Row-sharding falls out of shard_map: in_specs=P("core") hands device c
    rows [c*1024, (c+1)*1024) of the full array, which is exactly the
    per-core BIR-declared shape; the output concatenates the same way.
    """
    import jax
    from jax.experimental.shard_map import shard_map
    from jax.sharding import Mesh, PartitionSpec

    from concourse import bass2jax

    bass2jax.install_neuronx_cc_hook()
    out_avals = (
        jax.core.ShapedArray((ROWS_PER_CORE, DIM), jax.numpy.bfloat16),
    )
    partition_name = (
        nc.partition_id_tensor.name if nc.partition_id_tensor else None
    )
    in_names = ("x", "y") + ((partition_name,) if partition_name else ())

    def _body(x, y_zero):
        operands = [x, y_zero]
        if partition_name:
            operands.append(bass2jax.partition_id_tensor())
        outs = bass2jax._bass_exec_p.bind(
            *operands,
            out_avals=out_avals,
            in_names=in_names,
            out_names=("y",),
            lowering_input_output_aliases=(),
            sim_require_finite=True,
            sim_require_nnan=True,
            nc=nc,
        )
        return outs[0]

    devices = jax.devices()[:N_CORES]
    assert len(devices) == N_CORES
    mesh = Mesh(np.asarray(devices), ("core",))
    spec = PartitionSpec("core")
    sharding = jax.sharding.NamedSharding(mesh, spec)
    fn = jax.jit(
        shard_map(
            _body,
            mesh=mesh,
            in_specs=(spec, spec),
            out_specs=spec,
            check_rep=False,
        ),
        donate_argnums=(1,) if donate else (),
        keep_unused=True,
    )
    # Donated zero output buffers, built on-device (the axon host->device
    # path is slow; 128 MB of host zeros per call would dominate runtime).
    zeros_fn = jax.jit(
        lambda: jax.numpy.zeros((DIM, DIM), jax.numpy.bfloat16),
        out_shardings=sharding,
    )
    return fn, zeros_fn


def _get_callable():
    if "fn" not in _CACHE:
        _CACHE["fn"] = _make_callable(_get_nc())
    return _CACHE["fn"]


def kernel(corr: np.ndarray) -> np.ndarray:
    import jax

    corr = np.ascontiguousarray(np.asarray(corr, dtype=np.float32))
    assert corr.shape == (DIM, DIM)

    try:
        fn, zeros_fn = _get_callable()
        out_bf16 = np.asarray(jax.block_until_ready(fn(corr, zeros_fn())))
    except Exception:
        # Fallback: the stock (uncached) execution path.
        from concourse.bass_utils import run_bass_kernel_spmd

        nc = _get_nc()
        in_maps = [
            {"x": corr[c * ROWS_PER_CORE : (c + 1) * ROWS_PER_CORE]}
            for c in range(N_CORES)
        ]
        res = run_bass_kernel_spmd(nc, in_maps, list(range(N_CORES)))
        out_bf16 = np.concatenate(
            [res.results[c]["y"] for c in range(N_CORES)], axis=0
        )
    # Exact widening bf16 -> f32 on the host (output contract is f32).
    return out_bf16.astype(np.float32)


# revision 9
# speedup vs baseline: 1.0631x; 1.0631x over previous
"""Row L2-normalization kernel for Trainium2 (raw Bass), 8-core SPMD.

out[i, j] = corr[i, j] / sqrt(sum_j corr[i, j]^2)

Sharding: row-wise across 8 cores — each core owns a [1024, 8192] slab.
Row norms are fully row-local, so there is no cross-core communication.

The shipped build is `_build_nc_v3` (phase-separated schedule). Key
facts driving it, all measured on this hardware with DMA-only probe
kernels: each DMA direction alone sustains ~342 GB/s/core (loads-only
32 MB in 98.2 us, stores-only 16 MB in 49.1 us) while mixed
read+write streaming only gets ~317 GB/s/core. So the schedule
de-mixes the directions: per pass, the 8 tile loads (32 MB f32)
stream back-to-back with compute pipelined behind them into a 6-slot
bf16 output ring; stores are held until tile 5 is scaled (~90%
through the read phase) and then burst out back-to-back (16 MB); the
next pass's loads wait for the store burst to drain (the last 2
stores exempt — overlapping them with the first loads trades ~1 us
of mixing for ~12 us of serialized drain). One engine per stage:

    SP   : DMA load x -> t[g%3]             (4 MB tile loads)
    ACT  : Square(out=junk, accum_out=rowsum); Sqrt(rowsum)
    DVE  : reciprocal(rowsum); o = t * rowsum
           (tensor_scalar_mul, f32 -> bf16 downcast on write)
    POOL : DMA store o -> y                 (2 MB tile stores, burst)

The output is written to HBM as bf16 and widened to f32 on the host
(exact widening; the only loss is the bf16 rounding of the normalized
values — the normalized entries are all in [-1, 1] so bf16's 8-bit
mantissa bounds the per-element relative error at 2^-9; measured
Frobenius rel err 1.66e-3 vs the 2e-2 gate). This cuts per-core HBM
traffic from 64 MB (32 read + 32 write f32) to 48 MB (32 read f32 +
16 write bf16). A 1-byte output dtype (e4m3, ~3% RMS quantization)
would breach the 2e-2 gate, so 48 MB is the traffic floor.

Measured on HW via K-pass NEFFs (see test.py): 152.5-153.6 us/pass.
Lineage: f32-out streaming ~200 us -> bf16-out streaming (v1/v2)
158.3-160.4 us (within 1 us of its 157.6 us mixed-traffic DMA floor,
measured by a compute-free probe with identical transfers) -> this
phase-separated v3 at 152.5-153.6 us, approaching the 147.3 us
directional floor (98.2 + 49.1).

`_build_nc*(n_passes=K)` emits K back-to-back full passes in one NEFF
(every pass re-reads x from HBM and re-writes y); the semaphore
formulas are uniform in the global tile index so passes pipeline
identically. kernel() itself uses the 1-pass build; test.py uses a
K-pass build to measure per-pass device time with the dispatch
overhead amortized away.

`_build_nc` (v1: 8 streaming single-tile transfers, 3-deep buffers,
159.6-160.4 us) and `_build_nc_v2` (row-pair layout, 4+4 max-size
transfers, 158.3-159.1 us) are kept for reference.

Raw Bass (not Tile) because this walrus build rejects compute
instructions carrying >1 embedded semaphore wait; here every wait is a
standalone wait_ge.
"""

import sys

for _p in ("/opt/trn_rl_repo", "/root/.axon_site/_ro/trn_rl_repo"):
    if _p not in sys.path:
        sys.path.append(_p)

import numpy as np

DIM = 8192
N_CORES = 8
ROWS_PER_CORE = DIM // N_CORES  # 1024
P = 128
N_TILES = ROWS_PER_CORE // P  # 8
N_T_BUFS = 3
N_O_BUFS = 3

_CACHE: dict = {}


def _build_nc(n_passes: int = 1):
    import concourse.bass as bass
    from concourse import mybir

    nc = bass.Bass()
    f32 = mybir.dt.float32
    bf16 = mybir.dt.bfloat16
    x = nc.dram_tensor("x", [ROWS_PER_CORE, DIM], f32, kind="ExternalInput")
    y = nc.dram_tensor("y", [ROWS_PER_CORE, DIM], bf16, kind="ExternalOutput")
    xt = x.rearrange("(n p) m -> n p m", p=P)
    yt = y.rearrange("(n p) m -> n p m", p=P)

    TOTAL = n_passes * N_TILES

    with (
        nc.sbuf_tensor([P, N_T_BUFS, DIM], f32) as t_buf,
        nc.sbuf_tensor([P, N_O_BUFS, DIM], bf16) as o_buf,
        nc.sbuf_tensor([P, N_TILES], f32) as norms,
        nc.semaphore("t_sem0") as t_sem0,
        nc.semaphore("t_sem1") as t_sem1,
        nc.semaphore("t_sem2") as t_sem2,
        nc.semaphore("o_sem0") as o_sem0,
        nc.semaphore("o_sem1") as o_sem1,
        nc.semaphore("o_sem2") as o_sem2,
        nc.semaphore("act") as act_sem,
        nc.semaphore("dve") as dve_sem,
        nc.Block() as block,
    ):
        # One DMA semaphore per buffer slot: a DMA's 16 increments land
        # unordered across SDMA engines, so cumulative waits on a sem shared
        # by concurrent DMAs would be racy. Per slot, transfers serialize.
        t_sems = [t_sem0, t_sem1, t_sem2]
        o_sems = [o_sem0, o_sem1, o_sem2]

        @block.sync
        def _(sync):
            for i in range(TOTAL):
                if i >= N_T_BUFS:
                    # t-slot free once the DVE scale of tile i-3 has read it
                    sync.wait_ge(dve_sem, 2 * (i - N_T_BUFS) + 2)
                sync.dma_start(
                    out=t_buf[:, i % N_T_BUFS, :], in_=xt[i % N_TILES]
                ).then_inc(t_sems[i % N_T_BUFS], 16)

        @block.scalar
        def _(scalar):
            for i in range(TOTAL):
                t = t_buf[:, i % N_T_BUFS, :]
                o = o_buf[:, i % N_O_BUFS, :]
                norm = norms[:, i % N_TILES : i % N_TILES + 1]
                scalar.wait_ge(t_sems[i % N_T_BUFS], 16 * (i // N_T_BUFS + 1))
                if i >= N_O_BUFS:
                    # o-slot free once tile i-3's store has drained
                    scalar.wait_ge(o_sems[i % N_O_BUFS], 16 * (i // N_O_BUFS))
                # The Square's elementwise output is junk dumped into the
                # o-tile (the DVE scale overwrites it); only accum_out is
                # consumed.
                scalar.activation(
                    out=o,
                    in_=t,
                    func=mybir.ActivationFunctionType.Square,
                    accum_out=norm,
                ).then_inc(act_sem, 1)
                # ACT pipelines back-to-back instructions; the accum_out
                # write lands at completion, so same-engine RAW needs a wait.
                scalar.wait_ge(act_sem, 2 * i + 1)
                scalar.sqrt(out=norm, in_=norm).then_inc(act_sem, 1)

        HALF = DIM // 2
        LAST = TOTAL - 1

        @block.vector
        def _(vector):
            for i in range(TOTAL):
                t = t_buf[:, i % N_T_BUFS, :]
                o = o_buf[:, i % N_O_BUFS, :]
                norm = norms[:, i % N_TILES : i % N_TILES + 1]
                # sqrt done => square done => load i landed (sem values fire
                # at instruction completion, so this transitivity is sound)
                vector.wait_ge(act_sem, 2 * i + 2)
                vector.reciprocal(out=norm, in_=norm).then_inc(dve_sem, 1)
                vector.wait_ge(dve_sem, 2 * i + 1)
                if i < LAST:
                    vector.tensor_scalar_mul(o, t, norm).then_inc(dve_sem, 1)
                else:
                    # Last tile: scale in column halves so the first half-
                    # store overlaps the second half-scale (shorter tail).
                    vector.tensor_scalar_mul(
                        o[:, :HALF], t[:, :HALF], norm
                    ).then_inc(dve_sem, 1)
                    vector.wait_ge(dve_sem, 2 * i + 2)
                    vector.tensor_scalar_mul(
                        o[:, HALF:], t[:, HALF:], norm
                    ).then_inc(dve_sem, 1)

        @block.gpsimd
        def _(gpsimd):
            for i in range(TOTAL):
                o = o_buf[:, i % N_O_BUFS, :]
                yto = yt[i % N_TILES]
                gpsimd.wait_ge(dve_sem, 2 * i + 2)
                if i < LAST:
                    gpsimd.dma_start(out=yto, in_=o).then_inc(
                        o_sems[i % N_O_BUFS], 16
                    )
                else:
                    gpsimd.dma_start(
                        out=yto[:, :HALF], in_=o[:, :HALF]
                    ).then_inc(o_sems[i % N_O_BUFS], 16)
                    gpsimd.wait_ge(dve_sem, 2 * i + 3)
                    gpsimd.dma_start(
                        out=yto[:, HALF:], in_=o[:, HALF:]
                    ).then_inc(o_sems[i % N_O_BUFS], 16)

    return nc


def _build_nc_v2(n_passes: int = 1):
    """Row-pair layout: partition p of pair-tile j holds DRAM rows
    256*j + 2p and 256*j + 2p + 1, so each partition's load line is
    64 KB contiguous DRAM (one max-size SDMA descriptor) and each
    store line is 32 KB contiguous. A pass is 4 pair-tiles: 4 loads
    of 8 MB + 4 stores of 4 MB instead of 8+8 half-size transfers —
    half the per-transfer fixed costs (DGE issue + semaphore
    propagation) that dominate the gap to the bandwidth roofline.

    The two rows sharing a partition are normalized independently:
    the ACT Square/accum runs once per 8192-column sub-block, giving
    one row-norm per partition per sub-block.
    """
    import concourse.bass as bass
    from concourse import mybir

    nc = bass.Bass()
    f32 = mybir.dt.float32
    bf16 = mybir.dt.bfloat16
    x = nc.dram_tensor("x", [ROWS_PER_CORE, DIM], f32, kind="ExternalInput")
    y = nc.dram_tensor("y", [ROWS_PER_CORE, DIM], bf16, kind="ExternalOutput")
    # pair-tile j, partition p, sub-block s <-> DRAM row 256j + 2p + s
    xt = x.rearrange("(n p two) m -> n p (two m)", p=P, two=2)
    yt = y.rearrange("(n p two) m -> n p (two m)", p=P, two=2)

    PAIRS = N_TILES // 2  # 4 pair-tiles per pass
    W2 = 2 * DIM
    TOTAL = n_passes * PAIRS
    N_SLOTS = 2

    with (
        nc.sbuf_tensor([P, N_SLOTS, W2], f32) as t_buf,
        nc.sbuf_tensor([P, N_SLOTS, W2], bf16) as o_buf,
        nc.sbuf_tensor([P, 2 * PAIRS], f32) as norms,
        nc.semaphore("t_sem0") as t_sem0,
        nc.semaphore("t_sem1") as t_sem1,
        nc.semaphore("o_sem0") as o_sem0,
        nc.semaphore("o_sem1") as o_sem1,
        nc.semaphore("act") as act_sem,
        nc.semaphore("dve") as dve_sem,
        nc.Block() as block,
    ):
        t_sems = [t_sem0, t_sem1]
        o_sems = [o_sem0, o_sem1]
        LAST = TOTAL - 1

        @block.sync
        def _(sync):
            for j in range(TOTAL):
                if j >= N_SLOTS:
                    # t-slot free once both DVE scales of pair j-2 read it
                    sync.wait_ge(dve_sem, 4 * (j - N_SLOTS) + 4)
                sync.dma_start(
                    out=t_buf[:, j % N_SLOTS, :], in_=xt[j % PAIRS]
                ).then_inc(t_sems[j % N_SLOTS], 16)

        @block.scalar
        def _(scalar):
            for j in range(TOTAL):
                t = t_buf[:, j % N_SLOTS, :]
                o = o_buf[:, j % N_SLOTS, :]
                scalar.wait_ge(t_sems[j % N_SLOTS], 16 * (j // N_SLOTS + 1))
                if j >= N_SLOTS:
                    # o-slot free once pair j-2's store has drained
                    scalar.wait_ge(o_sems[j % N_SLOTS], 16 * (j // N_SLOTS))
                for s in (0, 1):
                    c = (j % PAIRS) * 2 + s
                    norm = norms[:, c : c + 1]
                    sub = slice(s * DIM, (s + 1) * DIM)
                    # Square's elementwise output is junk dumped into the
                    # o sub-block (the DVE scale overwrites it).
                    scalar.activation(
                        out=o[:, sub],
                        in_=t[:, sub],
                        func=mybir.ActivationFunctionType.Square,
                        accum_out=norm,
                    ).then_inc(act_sem, 1)
                    scalar.wait_ge(act_sem, 4 * j + 2 * s + 1)
                    scalar.sqrt(out=norm, in_=norm).then_inc(act_sem, 1)

        @block.vector
        def _(vector):
            for j in range(TOTAL):
                t = t_buf[:, j % N_SLOTS, :]
                o = o_buf[:, j % N_SLOTS, :]
                for s in (0, 1):
                    c = (j % PAIRS) * 2 + s
                    norm = norms[:, c : c + 1]
                    sub = slice(s * DIM, (s + 1) * DIM)
                    vector.wait_ge(act_sem, 4 * j + 2 * s + 2)
                    vector.reciprocal(out=norm, in_=norm).then_inc(dve_sem, 1)
                    vector.wait_ge(dve_sem, 4 * j + 2 * s + 1)
                    vector.tensor_scalar_mul(
                        o[:, sub], t[:, sub], norm
                    ).then_inc(dve_sem, 1)

        @block.gpsimd
        def _(gpsimd):
            for j in range(TOTAL):
                o = o_buf[:, j % N_SLOTS, :]
                yto = yt[j % PAIRS]
                if j < LAST:
                    gpsimd.wait_ge(dve_sem, 4 * j + 4)
                    gpsimd.dma_start(out=yto, in_=o).then_inc(
                        o_sems[j % N_SLOTS], 16
                    )
                else:
                    # Last pair: store per sub-block so the first store
                    # overlaps the second scale (shorter tail).
                    gpsimd.wait_ge(dve_sem, 4 * j + 2)
                    gpsimd.dma_start(
                        out=yto[:, :DIM], in_=o[:, :DIM]
                    ).then_inc(o_sems[j % N_SLOTS], 16)
                    gpsimd.wait_ge(dve_sem, 4 * j + 4)
                    gpsimd.dma_start(
                        out=yto[:, DIM:], in_=o[:, DIM:]
                    ).then_inc(o_sems[j % N_SLOTS], 16)

    return nc


def _build_nc_v3(n_passes: int = 1):
    """Phase-separated schedule: pure-read phase, then pure-write phase.

    Directional DMA probes show each direction alone sustains
    ~342 GB/s/core while mixed read+write traffic only gets
    ~317 GB/s/core. This schedule de-mixes them: all 8 tile loads
    stream first (32 MB, compute pipelined behind them into a 6-slot
    bf16 output ring), stores are held back until tile 5 is scaled
    (~90% through the read phase) and then burst out back-to-back
    (16 MB), and the next pass's loads wait for the store burst to
    drain. Expected ~147-150 us/pass vs ~158.5 for the streaming v2.

    SBUF: 3 f32 load slots (96 KB/partition) + 6 bf16 output slots
    (96 KB/partition) = 192 KB of the ~208 KB budget. Six output slots
    are enough because stores for tiles 0..5 drain while tiles 6, 7
    are still being squared/scaled.
    """
    import concourse.bass as bass
    from concourse import mybir

    nc = bass.Bass()
    f32 = mybir.dt.float32
    bf16 = mybir.dt.bfloat16
    x = nc.dram_tensor("x", [ROWS_PER_CORE, DIM], f32, kind="ExternalInput")
    y = nc.dram_tensor("y", [ROWS_PER_CORE, DIM], bf16, kind="ExternalOutput")
    xt = x.rearrange("(n p) m -> n p m", p=P)
    yt = y.rearrange("(n p) m -> n p m", p=P)

    TOTAL = n_passes * N_TILES
    NT = 3  # f32 load slots
    NO = 6  # bf16 output-ring slots

    def stores_done_before(g0):
        """o_sems thresholds guaranteeing every store with g < g0 drained."""
        return [16 * len([g for g in range(g0) if g % NO == s]) for s in range(NO)]

    with (
        nc.sbuf_tensor([P, NT, DIM], f32) as t_buf,
        nc.sbuf_tensor([P, NO, DIM], bf16) as o_buf,
        nc.sbuf_tensor([P, N_TILES], f32) as norms,
        nc.semaphore("t_sem0") as t_sem0,
        nc.semaphore("t_sem1") as t_sem1,
        nc.semaphore("t_sem2") as t_sem2,
        nc.semaphore("o_sem0") as o_sem0,
        nc.semaphore("o_sem1") as o_sem1,
        nc.semaphore("o_sem2") as o_sem2,
        nc.semaphore("o_sem3") as o_sem3,
        nc.semaphore("o_sem4") as o_sem4,
        nc.semaphore("o_sem5") as o_sem5,
        nc.semaphore("act") as act_sem,
        nc.semaphore("dve") as dve_sem,
        nc.Block() as block,
    ):
        t_sems = [t_sem0, t_sem1, t_sem2]
        o_sems = [o_sem0, o_sem1, o_sem2, o_sem3, o_sem4, o_sem5]
        LAST = TOTAL - 1

        @block.sync
        def _(sync):
            for g in range(TOTAL):
                i = g % N_TILES
                if i == 0 and g > 0:
                    # Inter-pass barrier: the previous pass's store burst
                    # must drain before this pass's reads begin, so reads
                    # and writes stay unmixed. The last 2 stores are
                    # exempt: letting them overlap the first loads trades
                    # ~1 us of mixing for ~12 us of serialized drain.
                    for s, thr in enumerate(stores_done_before(g - 2)):
                        if thr:
                            sync.wait_ge(o_sems[s], thr)
                if g >= NT:
                    # t-slot free once the DVE scale of tile g-3 has read it
                    sync.wait_ge(dve_sem, 2 * (g - NT) + 2)
                sync.dma_start(
                    out=t_buf[:, g % NT, :], in_=xt[i]
                ).then_inc(t_sems[g % NT], 16)

        @block.scalar
        def _(scalar):
            for g in range(TOTAL):
                t = t_buf[:, g % NT, :]
                o = o_buf[:, g % NO, :]
                norm = norms[:, g % N_TILES : g % N_TILES + 1]
                scalar.wait_ge(t_sems[g % NT], 16 * (g // NT + 1))
                if g >= NO:
                    # o-slot free once tile g-6's store has drained
                    scalar.wait_ge(o_sems[g % NO], 16 * (g // NO))
                scalar.activation(
                    out=o,
                    in_=t,
                    func=mybir.ActivationFunctionType.Square,
                    accum_out=norm,
                ).then_inc(act_sem, 1)
                scalar.wait_ge(act_sem, 2 * g + 1)
                scalar.sqrt(out=norm, in_=norm).then_inc(act_sem, 1)

        @block.vector
        def _(vector):
            for g in range(TOTAL):
                t = t_buf[:, g % NT, :]
                o = o_buf[:, g % NO, :]
                norm = norms[:, g % N_TILES : g % N_TILES + 1]
                vector.wait_ge(act_sem, 2 * g + 2)
                vector.reciprocal(out=norm, in_=norm).then_inc(dve_sem, 1)
                vector.wait_ge(dve_sem, 2 * g + 1)
                vector.tensor_scalar_mul(o, t, norm).then_inc(dve_sem, 1)

        @block.gpsimd
        def _(gpsimd):
            for g in range(TOTAL):
                i = g % N_TILES
                p = g // N_TILES
                o = o_buf[:, g % NO, :]
                # Hold tiles 0..5 back until tile 5 of this pass is scaled
                # (~90% through the read phase); tiles 6, 7 gate on their
                # own scale. The ring then bursts the stores back-to-back.
                gate = max(g, N_TILES * p + 5) if i <= 5 else g
                gpsimd.wait_ge(dve_sem, 2 * gate + 2)
                gpsimd.dma_start(out=yt[i], in_=o).then_inc(
                    o_sems[g % NO], 16
                )

    return nc


def _get_nc():
    if "nc" not in _CACHE:
        _CACHE["nc"] = _build_nc_v3()
    return _CACHE["nc"]


def _make_callable(nc, donate: bool = True):
    """Compile a Bass module into a sharded PJRT callable over 8 cores.

    Row-sharding falls out of shard_map: in_specs=P("core") hands device c
    rows [c*1024, (c+1)*1024) of the full array, which is exactly the
    per-core BIR-declared shape; the output concatenates the same way.
    """
    import jax
    from jax.experimental.shard_map import shard_map
    from jax.sharding import Mesh, PartitionSpec

    from concourse import bass2jax

    bass2jax.install_neuronx_cc_hook()
    out_avals = (
        jax.core.ShapedArray((ROWS_PER_CORE, DIM), jax.numpy.bfloat16),
    )
    partition_name = (
        nc.partition_id_tensor.name if nc.partition_id_tensor else None
    )
    in_names = ("x", "y") + ((partition_name,) if partition_name else ())

    def _body(x, y_zero):
        operands = [x, y_zero]
        if partition_name:
            operands.append(bass2jax.partition_id_tensor())
        outs = bass2jax._bass_exec_p.bind(
            *operands,
            out_avals=out_avals,
            in_names=in_names,
            out_names=("y",),
            lowering_input_output_aliases=(),
            sim_require_finite=True,
            sim_require_nnan=True,
            nc=nc,
        )
        return outs[0]

    devices = jax.devices()[:N_CORES]
    assert len(devices) == N_CORES
    mesh = Mesh(np.asarray(devices), ("core",))
    spec = PartitionSpec("core")
    sharding = jax.sharding.NamedSharding(mesh, spec)
    fn = jax.jit(
        shard_map(
            _body,
            mesh=mesh,
            in_specs=(spec, spec),
            out_specs=spec,
            check_rep=False,
        ),
        donate_argnums=(1,) if donate else (),
        keep_unused=True,
    )
    # Donated zero output buffers, built on-device (the axon host->device
    # path is slow; 128 MB of host zeros per call would dominate runtime).
    zeros_fn = jax.jit(
        lambda: jax.numpy.zeros((DIM, DIM), jax.numpy.bfloat16),
        out_shardings=sharding,
    )
    return fn, zeros_fn


def _get_callable():
    if "fn" not in _CACHE:
        _CACHE["fn"] = _make_callable(_get_nc())
    return _CACHE["fn"]


def kernel(corr: np.ndarray) -> np.ndarray:
    import jax

    corr = np.ascontiguousarray(np.asarray(corr, dtype=np.float32))
    assert corr.shape == (DIM, DIM)

    try:
        fn, zeros_fn = _get_callable()
        out_bf16 = np.asarray(jax.block_until_ready(fn(corr, zeros_fn())))
    except Exception:
        # Fallback: the stock (uncached) execution path.
        from concourse.bass_utils import run_bass_kernel_spmd

        nc = _get_nc()
        in_maps = [
            {"x": corr[c * ROWS_PER_CORE : (c + 1) * ROWS_PER_CORE]}
            for c in range(N_CORES)
        ]
        res = run_bass_kernel_spmd(nc, in_maps, list(range(N_CORES)))
        out_bf16 = np.concatenate(
            [res.results[c]["y"] for c in range(N_CORES)], axis=0
        )
    # Exact widening bf16 -> f32 on the host (output contract is f32).
    return out_bf16.astype(np.float32)
